# revision 1
# baseline (speedup 1.0000x reference)
"""Trainium2 Bass kernel for nn_Block_56538949484919 (dense transformer block).

Sharding: data-parallel over batch B=4 x 2-way split of the query rows
(sequence dim) => 8 cores, no collectives. Each core receives its batch's
h_shared / h_private pre-transposed to feature-major layout [C, L] with the
sequence axis rolled so that its query half is always columns [0, L/2).
K/V are computed over the full (rolled) sequence on every core; attention is
permutation-invariant over keys, so the roll does not change the result.

Host-side precomputation (cheap, O(C^2)):
  - Wvv = Wv @ Wvt (value transform folded into one matrix)
  - LayerNorm affine (w, b) folded into the following weight matrix/bias
  - all weights pre-packed into fp8 DoubleRow layouts (hi in e4m3; for the
    residual-sensitive matmuls a lo = W - fp8(W) correction in e5m2, whose
    subnormal range covers the tiny residuals).

Numerics strategy (gate is relmax < 2e-2; this lands ~3e-3):
  - All projection/MLP matmuls run as fp8e4m3 DoubleRow (2 contraction
    k-tiles per instruction at 0.5 cycles/row in the cost model).
  - Residual-sensitive GEMMs (Wcp, Wfc, Wproj) are 3-term hi/lo:
    W_hi*x_hi + W_lo*x_hi + W_hi*x_lo ~= W*x at ~bf16 accuracy.
  - Attention-internal GEMMs (V/K/Q, att@V) are single-fp8: their
    quantization noise is damped by softmax averaging over L=2048 keys.
  - Scores stay bf16 (64-wide contraction cannot DoubleRow); exp on ACT.
  - Residual stream h_private and all PSUM accumulation stay fp32.

LayerNorm: transposed-layout statistics (matmuls with the x-slice as the
stationary operand and a ones column moving -> [128, 1] sums, nearly free on
the PE; one PSUM accumulation group per full 2KB bank - PSUM start zeroes a
whole zero-region). For h_private the LN is folded into V's output
evacuation (per-partition rstd / mu*rstd scales), so V/K/Q matmuls start
right after the input DMA. For h_shared the normalized fp8 operand is
produced by DVE (x*rstd) and an ACT-engine fp8 convert.

Elementwise work is spread across DVE / ACT / GPSIMD(Pool) to keep the
attention phase ACT(exp)-bound and the MLP phase PE-bound.
"""

import math
import os
import sys

import numpy as np

for _p in ("/opt/trn_rl_repo", "/opt/pypackages"):
    if _p not in sys.path and os.path.isdir(_p):
        sys.path.append(_p)

# Problem dims (hardcoded per spec)
B, L, C, H = 4, 2048, 1024, 16
HD = C // H            # 64 head dim
NCORES = 8
EPS = 1e-5
P = 128                # partitions
NF = 512               # matmul moving free-dim tile
NCH = C // P           # 8 feature chunks
NLT = L // P           # 16 sequence tiles
LQ = L // 2            # 1024 query rows per core
NIC = LQ // NF         # 2 i-chunks
NPAIR = H // 2         # 8 head pairs
VW = HD + 1            # 65 = value cols + ones column (softmax denominator)
F4 = 4 * C             # 4096
NFC = F4 // P          # 32 fc chunks
SCL = 1.0 / math.sqrt(HD)
GELU_FUNC = "Gelu_apprx_tanh"  # swapped to "Tanh" in CoreSim validation
REPEAT = 1  # >1 only for device-time benchmarking (emits the body N times)

_CACHE = {}


def _build_bass():
    import concourse.bass as bass
    import concourse.mybir as mybir
    import concourse.tile as tile
    from concourse import bacc
    from concourse.bass import ts

    dt = mybir.dt
    f32, bf16 = dt.float32, dt.bfloat16
    AF = mybir.ActivationFunctionType
    OP = mybir.AluOpType

    nc = bacc.Bacc()

    hsT = nc.dram_tensor("hst", [C, L], bf16, kind="ExternalInput")
    xp8d = nc.dram_tensor("xp8", [NCH // 2, P, 2, L], dt.float8e4, kind="ExternalInput")
    hpR = nc.dram_tensor("hpr", [C, LQ], f32, kind="ExternalInput")
    cvv = nc.dram_tensor("cvv", [C], f32, kind="ExternalInput")
    f8 = dt.float8e4
    f8l = dt.float8e5
    DR = mybir.MatmulPerfMode.DoubleRow
    # fp8 DoubleRow layouts: [out_tile, partition(k), ktile_pair, 2, out_cols]
    wq8 = nc.dram_tensor("wq8", [NPAIR, P, NCH // 2, 2, P], f8, kind="ExternalInput")
    wk8 = nc.dram_tensor("wk8", [NPAIR, P, NCH // 2, 2, P], f8, kind="ExternalInput")
    # moving-operand layout for V: [partition(k), ktile_pair, 2, out_features]
    wvv8 = nc.dram_tensor("wvv8", [P, NCH // 2, 2, C], f8, kind="ExternalInput")
    wcp8 = nc.dram_tensor("wcp8", [NCH, P, NCH // 2, 2, P], f8, kind="ExternalInput")
    wcp8l = nc.dram_tensor("wcp8l", [NCH, P, NCH // 2, 2, P], f8l, kind="ExternalInput")
    wfc8 = nc.dram_tensor("wfc8", [NFC, P, NCH // 2, 2, P], f8, kind="ExternalInput")
    wfc8l = nc.dram_tensor("wfc8l", [NFC, P, NCH // 2, 2, P], f8l, kind="ExternalInput")
    wproj8 = nc.dram_tensor("wproj8", [NCH, P, NFC // 2, 2, P], f8, kind="ExternalInput")
    wproj8l = nc.dram_tensor("wproj8l", [NCH, P, NFC // 2, 2, P], f8l,
                             kind="ExternalInput")
    bq = nc.dram_tensor("bq", [C], f32, kind="ExternalInput")
    bk = nc.dram_tensor("bk", [C], f32, kind="ExternalInput")
    bvv = nc.dram_tensor("bvv", [C], f32, kind="ExternalInput")
    bcp = nc.dram_tensor("bcp", [C], f32, kind="ExternalInput")
    bfc = nc.dram_tensor("bfc", [F4], f32, kind="ExternalInput")
    bproj = nc.dram_tensor("bproj", [C], f32, kind="ExternalInput")
    outT = nc.dram_tensor("outt", [C, LQ], f32, kind="ExternalOutput")
    DBG = os.environ.get("KDBG") == "1"
    if DBG:
        dbg_bf = nc.dram_tensor("dbgbf", [6, P, LQ], bf16, kind="ExternalOutput")
        dbg_f = nc.dram_tensor("dbgf", [2, P, LQ], f32, kind="ExternalOutput")
        dbg8 = nc.dram_tensor("dbg8", [6, P, 2, LQ], f8, kind="ExternalOutput")
        dbgv = nc.dram_tensor("dbgv", [P, 2, H, VW], f8, kind="ExternalOutput")

    with tile.TileContext(nc) as tc:
        with (
            tc.tile_pool(name="consts", bufs=1) as consts,
            tc.tile_pool(name="dram", bufs=1, space="DRAM") as dram,
        ):
            # --- constants ---
            ones_bf = consts.tile([P, 1], bf16)
            nc.vector.memset(ones_bf, 1.0)
            eps_sb = consts.tile([P, 1], f32)
            nc.vector.memset(eps_sb, EPS)
            ones64 = consts.tile([1, HD], bf16)
            nc.vector.memset(ones64, 1.0)
            bq_sb = consts.tile([P, NCH], f32)
            nc.sync.dma_start(out=bq_sb, in_=bq.rearrange("(o p) -> p o", p=P))
            bk_sb = consts.tile([P, NCH], f32)
            nc.sync.dma_start(out=bk_sb, in_=bk.rearrange("(o p) -> p o", p=P))
            bcp_sb = consts.tile([P, NCH], f32)
            nc.sync.dma_start(out=bcp_sb, in_=bcp.rearrange("(o p) -> p o", p=P))
            bproj_sb = consts.tile([P, NCH], f32)
            nc.sync.dma_start(out=bproj_sb, in_=bproj.rearrange("(o p) -> p o", p=P))
            bfc_sb = consts.tile([P, NFC], f32)
            nc.sync.dma_start(out=bfc_sb, in_=bfc.rearrange("(o p) -> p o", p=P))
            bvv_bc = consts.tile([P, C], f32)
            cvv_bc = consts.tile([P, C], f32)

            # ---------- plain LN: transposed stats + apply ----------
            def plain_ln(xpairs, Lx, statpool, bcpool, pspool, sqpool, tag,
                         out8=None, conv="dveact"):
                """xpairs: NCH/2 SBUF bf16 pair tiles [P, 2, Lx]; normalized
                in place as x*rstd - mu*rstd. If out8 is given (NCH/2 fp8 pair
                tiles [P, 2, Lx]), the final subtract writes fp8 there instead
                (xpairs keep x*rstd).

                Stats are computed in transposed layout: matmuls with the
                x-slice as the stationary operand and a ones column as the
                moving operand produce [128, 1] per-position sums (nearly free
                on the PE: cost scales with output free size)."""
                nsl = Lx // P
                # One accumulation group per full PSUM bank: PSUM start=True
                # zeroes a whole 2KB zero-region, so interleaved sub-bank
                # groups would clobber each other. Single start on the very
                # first matmul of each bank, stop on the last.
                ps_sum = pspool.tile([P, NF], f32, tag="pssum", name=f"pss_{tag}")
                ps_sq = pspool.tile([P, NF], f32, tag="pssq", name=f"psq_{tag}")
                for cp in range(NCH // 2):
                    sq = sqpool.tile([P, 2, Lx], bf16, tag="sq", name=f"sq_{tag}_{cp}")
                    nc.vector.tensor_mul(sq, xpairs[cp], xpairs[cp])
                    for t in range(2):
                        c = 2 * cp + t
                        for s in range(nsl):
                            nc.tensor.matmul(
                                ps_sum[:, s:s + 1], xpairs[cp][:, t, ts(s, P)],
                                ones_bf,
                                start=(c == 0 and s == 0),
                                stop=(c == NCH - 1 and s == nsl - 1),
                                skip_group_check=True,
                            )
                            nc.tensor.matmul(
                                ps_sq[:, s:s + 1], sq[:, t, ts(s, P)], ones_bf,
                                start=(c == 0 and s == 0),
                                stop=(c == NCH - 1 and s == nsl - 1),
                                skip_group_check=True,
                            )
                # post: mu, rstd, mu*rstd on [P, nsl] tiles
                t = statpool.tile([P, 2, nsl], f32, tag="stat", name=f"t_{tag}")
                nc.scalar.activation(out=t[:, 0, :], in_=ps_sum[:, 0:nsl],
                                     func=AF.Copy, scale=1.0 / C)
                nc.vector.tensor_mul(t[:, 1, :], t[:, 0, :], t[:, 0, :])
                nc.vector.scalar_tensor_tensor(
                    out=t[:, 1, :], in0=ps_sq[:, 0:nsl], scalar=1.0 / C,
                    in1=t[:, 1, :], op0=OP.mult, op1=OP.subtract,
                )
                nc.scalar.activation(out=t[:, 1, :], in_=t[:, 1, :], func=AF.Sqrt,
                                     bias=eps_sb)
                nc.vector.reciprocal(t[:, 1, :], t[:, 1, :])      # rstd
                nc.vector.tensor_mul(t[:, 0, :], t[:, 0, :], t[:, 1, :])  # mu*rstd
                tb = statpool.tile([P, 2, nsl], bf16, tag="statb", name=f"tb_{tag}")
                nc.vector.tensor_copy(tb, t)
                # transpose-bounce through DRAM, then partition-broadcast
                ab_d = dram.tile([2, Lx], bf16, name=f"ab_d_{tag}")
                nc.sync.dma_start(out=ab_d.rearrange("a (s p) -> p a s", p=P),
                                  in_=tb)
                murs_bc = bcpool.tile([P, Lx], bf16, tag="abc", name=f"mursbc_{tag}")
                nc.sync.dma_start(out=murs_bc, in_=ab_d[0:1, :].broadcast_to([P, Lx]))
                rs_bc = bcpool.tile([P, Lx], bf16, tag="abc", name=f"rsbc_{tag}")
                nc.sync.dma_start(out=rs_bc, in_=ab_d[1:2, :].broadcast_to([P, Lx]))
                for cp in range(NCH // 2):
                    for t in range(2):
                        xs = xpairs[cp][:, t, :]
                        nc.vector.tensor_mul(xs, xs, rs_bc)
                        if out8 is None:
                            nc.vector.tensor_sub(xs, xs, murs_bc)
                        elif conv == "dveact" and cp < 2:
                            with nc.allow_low_precision(reason="fp8 ln output"):
                                nc.vector.tensor_sub(out8[cp][:, t, :], xs,
                                                     murs_bc)
                        else:
                            nc.vector.tensor_sub(xs, xs, murs_bc)
                    if out8 is not None:
                        if conv == "act" or (conv == "dveact" and cp >= 2):
                            for t in range(2):
                                nc.scalar.activation(out=out8[cp][:, t, :],
                                                     in_=xpairs[cp][:, t, :],
                                                     func=AF.Copy)
                return xpairs

            for _rep in range(REPEAT):
              _r = "" if REPEAT == 1 else f"r{_rep}"
              # long-lived pools, allocated in lifetime order (LIFO release)
              h1pool = tc.alloc_tile_pool(name="h1p" + _r, bufs=NCH)
              ytpool = tc.alloc_tile_pool(name="ytp" + _r, bufs=NCH // 2)
              wpool3 = tc.alloc_tile_pool(name="wt3" + _r, bufs=2 * NCH)

              with tc.tile_pool(name="lns8p" + _r, bufs=NCH // 2) as lns8p:
                  vnpool = tc.alloc_tile_pool(name="vnp" + _r, bufs=NLT // 2)
                  vn_tiles = []
                  lns8 = [lns8p.tile([P, 2, L], f8, tag="lns8", name=f"lns8_{cp}")
                          for cp in range(NCH // 2)]
                  with tc.tile_pool(name="xp8p" + _r, bufs=NCH // 2) as xp8p:
                      # ---------- phase 1+2: hp stats (LN folded into V's
                      # output scales) + LN of hs ----------
                      wvvpool = tc.alloc_tile_pool(name="wvvp" + _r, bufs=1)
                      with (
                          tc.tile_pool(name="lnps" + _r, bufs=NCH // 2) as lnps,
                          tc.tile_pool(name="sqp" + _r, bufs=2) as sqpool,
                          tc.tile_pool(name="statp" + _r, bufs=2) as statpool,
                          tc.tile_pool(name="bcp" + _r, bufs=4) as bcpool,
                          tc.tile_pool(name="psstat" + _r, bufs=2, space="PSUM") as pspool,
                      ):
                          xp_sb = []
                          for cp in range(NCH // 2):
                              xc = xp8p.tile([P, 2, L], f8, tag="xp8", name=f"xp_{cp}")
                              nc.sync.dma_start(out=xc, in_=xp8d[cp])
                              xp_sb.append(xc)
                          # prefetch the V weights while hp streams in
                          wvv_sb = wvvpool.tile([P, NCH // 2, 2, C], f8, tag="wvv",
                                                name="wvv_sb")
                          nc.sync.dma_start(out=wvv_sb, in_=wvv8[:, :, :, :])
                          nc.sync.dma_start(out=bvv_bc,
                                            in_=bvv[None, :].broadcast_to([P, C]))
                          nc.sync.dma_start(out=cvv_bc,
                                            in_=cvv[None, :].broadcast_to([P, C]))
                          # transposed stats for hp (no broadcast needed: the
                          # per-column scales land on V's output partitions)
                          nslh = L // P
                          pshs = pspool.tile([P, NF], f32, tag="pssum", name="pss_hp")
                          pshq = pspool.tile([P, NF], f32, tag="pssq", name="psq_hp")
                          for cp in range(NCH // 2):
                              sq = sqpool.tile([P, 2, L], bf16, tag="sq",
                                               name=f"sq_hp_{cp}")
                              for t_ in range(2):
                                  nc.scalar.activation(out=sq[:, t_, :],
                                                       in_=xp_sb[cp][:, t_, :],
                                                       func=AF.Square)
                              for t in range(2):
                                  c = 2 * cp + t
                                  for sl in range(nslh):
                                      nc.tensor.matmul(
                                          pshs[:, sl:sl + 1],
                                          xp_sb[cp][:, t, ts(sl, P)], ones_bf,
                                          start=(c == 0 and sl == 0),
                                          stop=(c == NCH - 1 and sl == nslh - 1),
                                          skip_group_check=True,
                                      )
                                      nc.tensor.matmul(
                                          pshq[:, sl:sl + 1], sq[:, t, ts(sl, P)],
                                          ones_bf,
                                          start=(c == 0 and sl == 0),
                                          stop=(c == NCH - 1 and sl == nslh - 1),
                                          skip_group_check=True,
                                      )
                          thp = statpool.tile([P, 2, nslh], f32, tag="stat",
                                              name="t_hp")
                          nc.scalar.activation(out=thp[:, 0, :], in_=pshs[:, 0:nslh],
                                               func=AF.Copy, scale=1.0 / C)
                          nc.vector.tensor_mul(thp[:, 1, :], thp[:, 0, :],
                                               thp[:, 0, :])
                          nc.vector.scalar_tensor_tensor(
                              out=thp[:, 1, :], in0=pshq[:, 0:nslh], scalar=1.0 / C,
                              in1=thp[:, 1, :], op0=OP.mult, op1=OP.subtract,
                          )
                          nc.scalar.activation(out=thp[:, 1, :], in_=thp[:, 1, :],
                                               func=AF.Sqrt, bias=eps_sb)
                          nc.vector.reciprocal(thp[:, 1, :], thp[:, 1, :])
                          nc.vector.tensor_mul(thp[:, 0, :], thp[:, 0, :],
                                               thp[:, 1, :])
                          hs_pairs = []
                          for cp in range(NCH // 2):
                              xc = lnps.tile([P, 2, L], bf16, tag="lnh", name=f"hs_{cp}")
                              nc.sync.dma_start(
                                  out=xc,
                                  in_=hsT[ts(cp, 2 * P), :].rearrange(
                                      "(t p) l -> p t l", p=P))
                              hs_pairs.append(xc)
                          plain_ln(hs_pairs, L, statpool, bcpool, pspool,
                                   sqpool, "hs" + _r, out8=lns8, conv="act")

                      # ---------- phase 3: V (fp8 DoubleRow on raw x, LN
                      # applied to the output rows) ----------
                      with (
                          tc.tile_pool(name="w2p" + _r, bufs=2) as w2pool,
                          tc.tile_pool(name="vps" + _r, bufs=2, space="PSUM") as vpspool,
                      ):
                          for lt in range(NLT):
                              vps = vpspool.tile([P, C], f32, tag="vps", name=f"vps_{lt}")
                              for cp in range(NCH // 2):
                                  for dn in range(C // NF):
                                      nc.tensor.matmul(
                                          vps[:, ts(dn, NF)],
                                          xp_sb[cp][:, :, ts(lt, P)],
                                          wvv_sb[:, cp, :, ts(dn, NF)],
                                          start=(cp == 0), stop=(cp == NCH // 2 - 1),
                                          perf_mode=DR,
                                      )
                              if lt % 2 == 0:
                                  vnp = vnpool.tile([P, 2, H, VW], f8, tag="vn",
                                                    name=f"vn_{lt // 2}")
                                  nc.vector.memset(vnp[:, :, :, HD:VW], 1.0)
                                  vn_tiles.append(vnp)
                              # w2 = cvv*murs - bvv; vn = vps*rstd - w2
                              w2 = w2pool.tile([P, C], f32, tag="w2", name=f"w2_{lt}")
                              nc.vector.scalar_tensor_tensor(
                                  out=w2, in0=cvv_bc, scalar=thp[:, 0, lt:lt + 1],
                                  in1=bvv_bc, op0=OP.mult, op1=OP.subtract,
                              )
                              with nc.allow_low_precision(reason="fp8 v"):
                                  nc.vector.scalar_tensor_tensor(
                                      out=vn_tiles[lt // 2][:, lt % 2, :, 0:HD],
                                      in0=vps.rearrange("p (h d) -> p h d", d=HD),
                                      scalar=thp[:, 1, lt:lt + 1],
                                      in1=w2.rearrange("p (h d) -> p h d", d=HD),
                                      op0=OP.mult, op1=OP.subtract,
                                  )
                      if DBG:
                          nc.sync.dma_start(out=dbg8[0], in_=lns8[0][:, :, 0:LQ])
                          nc.sync.dma_start(out=dbgv[:, :, :, :], in_=vn_tiles[0])
                      wvvpool.release()
                  # lnps/xp8 released here

                  # ---------- phase 4: per-pair K/Q + attention, pipelined ----------
                  # prefetch the Wcp hi/lo tiles so phase 5 starts immediately
                  wcp_sb = []
                  for oc in range(NCH):
                      wct = wpool3.tile([P, NCH // 2, 2, P], f8, tag="wt3",
                                        name=f"wcpt_{oc}")
                      nc.sync.dma_start(out=wct, in_=wcp8[oc])
                      wctl = wpool3.tile([P, NCH // 2, 2, P], f8l, tag="wt3",
                                         name=f"wcptl_{oc}")
                      nc.sync.dma_start(out=wctl, in_=wcp8l[oc])
                      wcp_sb.append((wct, wctl))
                  yT = []
                  with (
                      tc.tile_pool(name="wtkq" + _r, bufs=4) as wkqpool,
                      tc.tile_pool(name="kqt" + _r, bufs=2) as kqtpool,
                      tc.tile_pool(name="pup" + _r, bufs=4) as pupool,
                      tc.tile_pool(name="recp" + _r, bufs=2) as recpool,
                      tc.tile_pool(name="kqps" + _r, bufs=1, space="PSUM") as kqpspool,
                      tc.tile_pool(name="stps" + _r, bufs=2, space="PSUM") as stpool,
                      tc.tile_pool(name="yps" + _r, bufs=2, space="PSUM") as ypool,
                      tc.tile_pool(name="repps" + _r, bufs=1, space="PSUM") as reppool,
                  ):
                      for pr in range(NPAIR):
                          # K for this pair: four [P, NF] rounds through one psum slot
                          kt = kqtpool.tile([P, L], bf16, tag="kt", name=f"kt_{pr}")
                          wkt = wkqpool.tile([P, NCH // 2, 2, P], f8, tag="wtkq",
                                             name=f"wkt_{pr}")
                          nc.sync.dma_start(out=wkt, in_=wk8[pr])
                          for r in range(L // NF):
                              ps = kqpspool.tile([P, NF], f32, tag="kqps", name=f"kps_{pr}_{r}")
                              for cp in range(NCH // 2):
                                  nc.tensor.matmul(
                                      ps, wkt[:, cp, :, :], lns8[cp][:, :, ts(r, NF)],
                                      start=(cp == 0), stop=(cp == NCH // 2 - 1),
                                      perf_mode=DR,
                                  )
                              nc.vector.tensor_scalar_add(
                                  out=kt[:, ts(r, NF)], in0=ps, scalar1=bk_sb[:, pr:pr + 1])
                          # Q for this pair
                          qt = kqtpool.tile([P, LQ], bf16, tag="qt", name=f"qt_{pr}")
                          wqt = wkqpool.tile([P, NCH // 2, 2, P], f8, tag="wtkq",
                                             name=f"wqt_{pr}")
                          nc.sync.dma_start(out=wqt, in_=wq8[pr])
                          for r in range(NIC):
                              ps = kqpspool.tile([P, NF], f32, tag="kqps", name=f"qps_{pr}_{r}")
                              for cp in range(NCH // 2):
                                  nc.tensor.matmul(
                                      ps, wqt[:, cp, :, :], lns8[cp][:, :, ts(r, NF)],
                                      start=(cp == 0), stop=(cp == NCH // 2 - 1),
                                      perf_mode=DR,
                                  )
                              nc.vector.tensor_scalar_add(
                                  out=qt[:, ts(r, NF)], in0=ps, scalar1=bq_sb[:, pr:pr + 1])

                          if DBG and pr == NPAIR - 1:
                              kqtpool_dbg = (kt, qt)
                          if pr % 2 == 0:
                              yT.append((
                                  ytpool.tile([P, 2, LQ], f8, tag="yth",
                                              name=f"yth_{pr // 2}"),
                                  ytpool.tile([P, 2, LQ], f8l, tag="ytl",
                                              name=f"ytl_{pr // 2}"),
                              ))
                          yth = yT[pr // 2][0][:, pr % 2, :]
                          ytl = yT[pr // 2][1][:, pr % 2, :]
                          for ic in range(NIC):
                              ypsA = ypool.tile([VW, NF], f32, tag="yps",
                                                name=f"ypsA_{pr}_{ic}")
                              ypsB = ypool.tile([VW, NF], f32, tag="yps",
                                                name=f"ypsB_{pr}_{ic}")
                              for jp in range(NLT // 2):
                                  for hh, yps in ((0, ypsA), (1, ypsB)):
                                      # st layout: [p, j-parity, n]
                                      st = stpool.tile([P, 2, NF], f32, tag="st",
                                                       name=f"st_{pr}_{ic}_{jp}_{hh}")
                                      for tp in range(2):
                                          j = 2 * jp + tp
                                          nc.tensor.matmul(
                                              st[:, tp, :],
                                              kt[ts(hh, HD), ts(j, P)],
                                              qt[ts(hh, HD), ts(ic, NF)],
                                              start=True, stop=True)
                                      pu = pupool.tile([P, 2, NF], f8, tag="pu",
                                                       name=f"pu_{pr}_{ic}_{jp}_{hh}")
                                      nc.scalar.activation(out=pu, in_=st,
                                                           func=AF.Exp, scale=SCL)
                                      nc.tensor.matmul(
                                          yps, vn_tiles[jp][:, :, 2 * pr + hh, :],
                                          pu,
                                          start=(jp == 0),
                                          stop=(jp == NLT // 2 - 1),
                                          perf_mode=DR)
                              for hh, yps in ((0, ypsA), (1, ypsB)):
                                  rec = recpool.tile([1, NF], bf16, tag="rec",
                                                     name=f"rec_{pr}_{ic}_{hh}")
                                  with nc.allow_low_precision(
                                          reason="softmax denom reciprocal, bf16 ok"):
                                      nc.vector.reciprocal(rec, yps[HD:VW, :])
                                  rep = reppool.tile([HD, NF], f32, tag="rep",
                                                     name=f"rep_{pr}_{ic}_{hh}")
                                  nc.tensor.matmul(rep, ones64, rec, start=True, stop=True)
                                  rep_sb = recpool.tile([HD, NF], f32, tag="repsb",
                                                        name=f"repsb_{pr}_{ic}_{hh}")
                                  nc.vector.tensor_copy(rep_sb, rep)
                                  ytb = recpool.tile([P, NF], bf16, tag="ytb",
                                                     name=f"ytb_{pr}_{ic}_{hh}")
                                  nc.vector.tensor_mul(ytb[ts(hh, HD), :],
                                                       yps[0:HD, :], rep_sb)
                                  nc.vector.tensor_copy(
                                      yth[ts(hh, HD), ts(ic, NF)],
                                      ytb[ts(hh, HD), :])
                                  with nc.allow_low_precision(
                                          reason="fp8 lo residual"):
                                      nc.vector.tensor_sub(
                                          ytl[ts(hh, HD), ts(ic, NF)],
                                          ytb[ts(hh, HD), :],
                                          yth[ts(hh, HD), ts(ic, NF)])
                      if DBG:
                          nc.sync.dma_start(out=dbg_bf[3], in_=kqtpool_dbg[0][:, 0:LQ])
                          nc.sync.dma_start(out=dbg_bf[4], in_=kqtpool_dbg[1])
                          nc.sync.dma_start(out=dbg8[2], in_=yT[0][0])
                          pass
                  vnpool.release()
              # lnps (ln_hs) released here

              # ---------- phase 5: Wcp + residual ----------
              h1 = []
              with (
                  tc.tile_pool(name="resp" + _r, bufs=3) as respool,
                  tc.tile_pool(name="cps" + _r, bufs=2, space="PSUM") as cpool,
              ):
                  for oc in range(NCH):
                      wct, wctl = wcp_sb[oc]
                      cps = cpool.tile([P, LQ], f32, tag="cps", name=f"cps_{oc}")
                      for term in range(3):
                          wt_ = (wct, wctl, wct)[term]
                          ysel = (0, 0, 1)[term]
                          for pp in range(NCH // 2):
                              for ic in range(NIC):
                                  nc.tensor.matmul(
                                      cps[:, ts(ic, NF)], wt_[:, pp, :, :],
                                      yT[pp][ysel][:, :, ts(ic, NF)],
                                      start=(term == 0 and pp == 0),
                                      stop=(term == 2 and pp == NCH // 2 - 1),
                                      perf_mode=DR,
                                  )
                      hp_r = respool.tile([P, LQ], f32, tag="res", name=f"hpr_{oc}")
                      nc.sync.dma_start(out=hp_r, in_=hpR[ts(oc, P), :])
                      h1c = h1pool.tile([P, LQ], f32, tag="h1", name=f"h1_{oc}")
                      nc.vector.scalar_tensor_tensor(
                          out=h1c, in0=cps, scalar=bcp_sb[:, oc:oc + 1], in1=hp_r,
                          op0=OP.add, op1=OP.add,
                      )
                      h1.append(h1c)
              wpool3.release()
              ytpool.release()

              # ---------- phase 6: ln2 -> fp8 hi/lo pair tiles ----------
              with (
                  tc.tile_pool(name="ln2h" + _r, bufs=NCH // 2) as ln2hpool,
                  tc.tile_pool(name="ln2l" + _r, bufs=NCH // 2) as ln2lpool,
              ):
                  with (
                      tc.tile_pool(name="ln2b" + _r, bufs=NCH // 2) as ln2bpool,
                      tc.tile_pool(name="sqp2" + _r, bufs=2) as sqpool2,
                      tc.tile_pool(name="statp2" + _r, bufs=2) as statpool2,
                      tc.tile_pool(name="bcp2" + _r, bufs=2) as bcpool2,
                      tc.tile_pool(name="psstat2" + _r, bufs=2, space="PSUM") as pspool2,
                  ):
                      h1_bf = []
                      for cp in range(NCH // 2):
                          hb = ln2bpool.tile([P, 2, LQ], bf16, tag="ln2b",
                                             name=f"h1b_{cp}")
                          for t in range(2):
                              nc.vector.tensor_copy(hb[:, t, :], h1[2 * cp + t])
                          h1_bf.append(hb)
                      ln2 = plain_ln(h1_bf, LQ, statpool2, bcpool2, pspool2,
                                     sqpool2, "l2" + _r)
                      # hi/lo fp8 pair tiles for 3-term DoubleRow fc
                      ln2h, ln2l = [], []
                      for cp in range(NCH // 2):
                          lh = ln2hpool.tile([P, 2, LQ], f8, tag="ln2h", name=f"ln2h_{cp}")
                          ll = ln2lpool.tile([P, 2, LQ], f8l, tag="ln2l", name=f"ln2l_{cp}")
                          nc.scalar.activation(out=lh, in_=ln2[cp], func=AF.Copy)
                          with nc.allow_low_precision(reason="fp8 lo residual"):
                              nc.vector.tensor_sub(ll, ln2[cp], lh)
                          ln2h.append(lh)
                          ln2l.append(ll)
                      if DBG:
                          nc.sync.dma_start(out=dbg_f[0], in_=h1[0])
                          nc.sync.dma_start(out=dbg8[4], in_=ln2h[0])
                          nc.sync.dma_start(out=dbg_bf[1], in_=ln2[0][:, 0, :])
                          nc.sync.dma_start(out=dbg_bf[2], in_=ln2[0][:, 1, :])

                  # ---------- phase 7: MLP (fp8 DoubleRow, 3-term hi/lo) ----------
                  with (
                      tc.tile_pool(name="fchp" + _r, bufs=NFC // 2) as fchpool,
                      tc.tile_pool(name="fclp" + _r, bufs=NFC // 2) as fclpool,
                  ):
                      fc8h, fc8l = [], []
                      wpool5 = tc.alloc_tile_pool(name="wt5" + _r, bufs=2)
                      wproj_pre = {}
                      with (
                          tc.tile_pool(name="wt4" + _r, bufs=4) as wpool4,
                          tc.tile_pool(name="fcb" + _r, bufs=3) as fcbpool,
                          tc.tile_pool(name="fps" + _r, bufs=3, space="PSUM") as fpool,
                      ):
                          # prefetch the first wproj tiles during fc
                          wpt0 = wpool5.tile([P, NFC // 2, 2, P], f8, tag="wt5",
                                             name="wprt_0")
                          nc.sync.dma_start(out=wpt0, in_=wproj8[0])
                          wptl0 = wpool5.tile([P, NFC // 2, 2, P], f8l, tag="wt5",
                                              name="wprtl_0")
                          nc.sync.dma_start(out=wptl0, in_=wproj8l[0])
                          wproj_pre[0] = (wpt0, wptl0)
                          for fo in range(NFC):
                              wft = wpool4.tile([P, NCH // 2, 2, P], f8, tag="wt4",
                                                name=f"wfct_{fo}")
                              nc.sync.dma_start(out=wft, in_=wfc8[fo])
                              wftl = wpool4.tile([P, NCH // 2, 2, P], f8l, tag="wt4",
                                                 name=f"wfctl_{fo}")
                              nc.sync.dma_start(out=wftl, in_=wfc8l[fo])
                              fps = fpool.tile([P, LQ], f32, tag="fps", name=f"fps_{fo}")
                              for term, (wt_, act_) in enumerate(
                                      ((wft, ln2h), (wftl, ln2h), (wft, ln2l))):
                                  for cp in range(NCH // 2):
                                      for ic in range(NIC):
                                          nc.tensor.matmul(
                                              fps[:, ts(ic, NF)], wt_[:, cp, :, :],
                                              act_[cp][:, :, ts(ic, NF)],
                                              start=(term == 0 and cp == 0),
                                              stop=(term == 2 and cp == NCH // 2 - 1),
                                              perf_mode=DR,
                                          )
                              if fo % 2 == 0:
                                  fc8h.append(fchpool.tile([P, 2, LQ], f8, tag="fch",
                                                           name=f"fch_{fo // 2}"))
                                  fc8l.append(fclpool.tile([P, 2, LQ], f8l, tag="fcl",
                                                           name=f"fcl_{fo // 2}"))
                              fcb = fcbpool.tile([P, LQ], bf16, tag="fcb",
                                                 name=f"fcb_{fo}")
                              nc.scalar.activation(out=fcb, in_=fps,
                                                   func=getattr(AF, GELU_FUNC),
                                                   bias=bfc_sb[:, fo:fo + 1])
                              with nc.allow_low_precision(reason="fp8 hi/lo"):
                                  nc.gpsimd.tensor_copy(fc8h[fo // 2][:, fo % 2, :], fcb)
                                  nc.vector.tensor_sub(fc8l[fo // 2][:, fo % 2, :], fcb,
                                                       fc8h[fo // 2][:, fo % 2, :])
                              if DBG and fo == 0:
                                  nc.sync.dma_start(out=dbg_bf[0], in_=fcb)

                      with (
                          tc.tile_pool(name="outp" + _r, bufs=3) as opool,
                          tc.tile_pool(name="pps" + _r, bufs=2, space="PSUM") as ppool,
                      ):
                          for oc in range(NCH):
                              if oc in wproj_pre:
                                  wpt, wptl = wproj_pre[oc]
                              else:
                                  wpt = wpool5.tile([P, NFC // 2, 2, P], f8, tag="wt5",
                                                    name=f"wprt_{oc}")
                                  nc.sync.dma_start(out=wpt, in_=wproj8[oc])
                                  wptl = wpool5.tile([P, NFC // 2, 2, P], f8l,
                                                     tag="wt5", name=f"wprtl_{oc}")
                                  nc.sync.dma_start(out=wptl, in_=wproj8l[oc])
                              pps = ppool.tile([P, LQ], f32, tag="pps", name=f"pps_{oc}")
                              for term, (wt_, act_) in enumerate(
                                      ((wpt, fc8h), (wptl, fc8h), (wpt, fc8l))):
                                  for fp in range(NFC // 2):
                                      for ic in range(NIC):
                                          nc.tensor.matmul(
                                              pps[:, ts(ic, NF)], wt_[:, fp, :, :],
                                              act_[fp][:, :, ts(ic, NF)],
                                              start=(term == 0 and fp == 0),
                                              stop=(term == 2 and fp == NFC // 2 - 1),
                                              perf_mode=DR,
                                          )
                              osb = opool.tile([P, LQ], f32, tag="osb", name=f"osb_{oc}")
                              nc.vector.scalar_tensor_tensor(
                                  out=osb, in0=pps, scalar=bproj_sb[:, oc:oc + 1], in1=h1[oc],
                                  op0=OP.add, op1=OP.add,
                              )
                              nc.sync.dma_start(out=outT[ts(oc, P), :], in_=osb)
                      wpool5.release()
              h1pool.release()

    nc.finalize()
    return nc


def _host_prep(inputs):
    """Fold weights on host; returns dict of shared (per-core-identical) arrays."""
    import ml_dtypes

    f64 = np.float64
    g = {k: np.asarray(v) for k, v in inputs.items()}
    Wv = g["Wv"].astype(f64)
    Wvt = g["Wvt"].astype(f64)
    Wvv = Wv @ Wvt
    bvv = g["bv"].astype(f64) @ Wvt + g["bvt"].astype(f64)

    def fold(w_ln, b_ln, W, bias):
        W = W.astype(f64)
        Wf = w_ln.astype(f64)[:, None] * W
        bf = b_ln.astype(f64) @ W + bias.astype(f64)
        return Wf, bf

    Wq_, bq_ = fold(g["ln1s_w"], g["ln1s_b"], g["Wq"], g["bq"])
    Wk_, bk_ = fold(g["ln1s_w"], g["ln1s_b"], g["Wk"], g["bk"])
    Wvv_, bvv_ = fold(g["ln1p_w"], g["ln1p_b"], Wvv, bvv)
    Wfc_, bfc_ = fold(g["ln2_w"], g["ln2_b"], g["Wfc"], g["bfc"])

    f8 = ml_dtypes.float8_e4m3

    f8e5 = ml_dtypes.float8_e5m2

    def dr_layout(W, d=None):
        """[K, N] -> [N/128, 128(k), K/256, 2, 128(n)] fp8 DoubleRow tiles."""
        K, N = W.shape
        Wt = W.reshape(K // 256, 2, 128, N // 128, 128)  # cp, t, k, ot, n
        return np.ascontiguousarray(Wt.transpose(3, 2, 0, 1, 4).astype(d or f8))

    def mv_layout(W):
        """[K, N] -> [128(k), K/256, 2, N] fp8 DoubleRow moving layout."""
        K, N = W.shape
        Wt = W.reshape(K // 256, 2, 128, N)  # cp, t, k, n
        return np.ascontiguousarray(Wt.transpose(2, 0, 1, 3).astype(f8))

    def lo(W):
        return W - W.astype(f8).astype(f64)

    Wcp_ = g["Wcp"].astype(f64)
    Wproj_ = g["Wproj"].astype(f64)
    cvv_ = Wvv_.sum(axis=0)
    return {
        "cvv": cvv_.astype(np.float32),
        "wq8": dr_layout(Wq_),
        "wk8": dr_layout(Wk_),
        "wvv8": mv_layout(Wvv_),
        "wcp8": dr_layout(Wcp_),
        "wcp8l": dr_layout(lo(Wcp_), f8e5),
        "wfc8": dr_layout(Wfc_),
        "wfc8l": dr_layout(lo(Wfc_), f8e5),
        "wproj8": dr_layout(Wproj_),
        "wproj8l": dr_layout(lo(Wproj_), f8e5),
        "bq": bq_.astype(np.float32),
        "bk": bk_.astype(np.float32),
        "bvv": bvv_.astype(np.float32),
        "bcp": g["bcp"].astype(np.float32),
        "bfc": bfc_.astype(np.float32),
        "bproj": g["bproj"].astype(np.float32),
    }


def kernel(**inputs):
    from concourse.bass_utils import run_bass_kernel_spmd

    attn_dim = int(np.asarray(inputs["attn_dim"]))
    assert attn_dim in (-2, 1), f"unsupported attn_dim {attn_dim}"

    h_private = np.asarray(inputs["h_private"], dtype=np.float32)
    h_shared = np.asarray(inputs["h_shared"], dtype=np.float32)

    shared_ins = _host_prep(inputs)

    import ml_dtypes

    bf16 = ml_dtypes.bfloat16
    f8 = ml_dtypes.float8_e4m3
    in_maps = []
    for core in range(NCORES):
        b, s = divmod(core, 2)
        roll = s * LQ
        hs = np.concatenate([h_shared[b][roll:], h_shared[b][:roll]], axis=0)
        hp = np.concatenate([h_private[b][roll:], h_private[b][:roll]], axis=0)
        m = dict(shared_ins)
        m["hst"] = np.ascontiguousarray(hs.T.astype(bf16))
        m["xp8"] = np.ascontiguousarray(
            hp.T.reshape(NCH // 2, 2, P, L).transpose(0, 2, 1, 3).astype(f8))
        m["hpr"] = np.ascontiguousarray(h_private[b][s * LQ:(s + 1) * LQ].T)
        in_maps.append(m)

    if "nc" not in _CACHE:
        _CACHE["nc"] = _build_bass()
    nc = _CACHE["nc"]

    res = run_bass_kernel_spmd(nc, in_maps, list(range(NCORES)))
    _CACHE["last_res"] = res

    out = np.empty((B, L, C), np.float32)
    for core in range(NCORES):
        b, s = divmod(core, 2)
        out[b, s * LQ:(s + 1) * LQ, :] = res.results[core]["outt"].T
    return out, h_shared

